# revision 1
# baseline (speedup 1.0000x reference)
"""Trainium2 Bass kernel for the KinematicBicycle rollout (H=8192) — v2.1.

kernel(x0, U, dt) -> [8193, 4] float32 trajectory, computed on TRN2.

Same algorithm as the baseline (chunked clamp-scan for speed, prefix sums
for theta/x/y, range reduction + ACT Sin), restructured for latency:
  - speed-scan runs in w' = v/dt units so the accel clip + scan-input fill
    fuse into ONE scalar_tensor_tensor op
  - range reduction is cody_waite_cascade (exact 3-term) + add_range_wrap
    for the +pi/2 branch: 4 V ops total instead of 7
  - x0-derived offsets (x00/y00/th0) are folded into the tri-matmuls as an
    accumulated ones_row x xrow term — no broadcast matmul, no ob add
  - th_in fused: (sg + offg) - g in one scalar_tensor_tensor
  - light parallel work (d-mult, small copies, g0) goes to GpSimd; Scalar
    runs the Sin/Copy/Identity chain; Vector keeps the critical scans
The rollout is a single sequential recurrence; the program is replicated
SPMD on all 8 cores and core 0's output is returned.
"""
import os
import numpy as np

import concourse.bacc as bacc
import concourse.bass as bass
import concourse.mybir as mybir
import concourse.tile as tile
from concourse.bass_utils import run_bass_kernel_spmd

F32 = mybir.dt.float32
OP = mybir.AluOpType
AF = mybir.ActivationFunctionType

H, P, C = 8192, 128, 64
L = 2.7
BIG = 1e30
HPI = float(np.pi / 2)
PI = float(np.pi)
TWOPI = float(2.0 * np.pi)
MAGIC = 12582912.0          # 1.5*2^23: fp32 round-to-nearest via add/sub
INV2PI = float(1.0 / (2.0 * np.pi))
# Cody-Waite split of 2*pi: c1 exact in fp32, c2 fp32, c3 the remainder.
CW1 = 6.28125
CW2 = float(np.float32(2.0 * np.pi - 6.28125))
CW3 = float(2.0 * np.pi - 6.28125 - float(np.float32(2.0 * np.pi - 6.28125)))
N_CORES = int(os.environ.get("KB_CORES", "8"))

LAST_RUN_INFO = {}
_CACHE = {}


def _maybe_disable_io_dge():
    if os.environ.get("KB_NODGE", "0") != "1":
        return
    import libneuronxla.libncc as ncc
    from concourse.compiler_utils import set_compiler_flags
    flags = list(ncc.NEURON_CC_FLAGS)
    out, i = [], 0
    while i < len(flags):
        f = flags[i]
        if f == "--internal-enable-dge-levels":
            out.append(f); i += 1
            while i < len(flags) and not flags[i].startswith("--"):
                if flags[i] != "io":
                    out.append(flags[i])
                i += 1
            continue
        if f == "--internal-disable-dge-levels":
            out.append(f); i += 1
            out.append("io")
            while i < len(flags) and not flags[i].startswith("--"):
                out.append(flags[i]); i += 1
            continue
        out.append(f); i += 1
    set_compiler_flags(out)


def _build(dt_val):
    nc = bacc.Bacc("TRN2", target_bir_lowering=False, debug=False)

    dt_f = float(dt_val)
    VMAXW = 30.0 / dt_f          # speed upper bound in w' = v/dt units
    NDT2 = -dt_f * dt_f          # -dt^2 (w' -> v*dt folding)

    x0_d = nc.dram_tensor("x0", [4], F32, kind="ExternalInput")
    U_d = nc.dram_tensor("U", [H, 2], F32, kind="ExternalInput")
    out_d = nc.dram_tensor("out", [H + 1, 4], F32, kind="ExternalOutput")

    NCH = 2 * (P + 1)
    with tile.TileContext(nc) as tc:
        with (
            tc.tile_pool(name="sb", bufs=1) as sb,
            tc.tile_pool(name="ps", bufs=1, space="PSUM") as ps,
        ):
            # ---- input DMAs (both on Sync; U first, it gates everything) --
            Ut = sb.tile([P, 2 * C], F32, tag="Ut")
            HH = H // 2
            nc.sync.dma_start(out=Ut, in_=U_d[:].rearrange("(p j) c -> p (j c)", p=P))
            xrow = sb.tile([1, 8], F32, tag="xrow")
            nc.sync.dma_start(out=xrow[0:1, 0:4],
                              in_=x0_d[:].rearrange("(o a) -> o a", o=1))

            # ---- constants (GpSimd memsets, overlap the DMA window) ------
            zero_b = sb.tile([P, 1], F32, tag="zero_b")
            nc.gpsimd.memset(zero_b, 0.0)
            hpi_b = sb.tile([P, 1], F32, tag="hpi_b")
            nc.gpsimd.memset(hpi_b, HPI)
            threes = sb.tile([P, C], F32, tag="threes")
            nc.gpsimd.memset(threes, 3.0)
            d0v = sb.tile([P, 2 * C], F32, tag="d0v")
            nc.gpsimd.memset(d0v, 0.0)
            d1v = sb.tile([P, 2 * C], F32, tag="d1v")
            nc.gpsimd.memset(d1v, 0.0)
            nc.gpsimd.memset(d1v[:, 1:2 * C:2], -VMAXW)
            stage65 = sb.tile([P, 65], F32, tag="stage65")
            nc.gpsimd.memset(stage65, 0.0)
            big = sb.tile([P, 160], F32, tag="big")
            d1c = sb.tile([1, NCH], F32, tag="d1c")
            nc.gpsimd.memset(d1c, -BIG)
            one_t = sb.tile([1, 1], F32, tag="one_t")
            nc.gpsimd.memset(one_t, 1.0)
            ones_row = sb.tile([1, P], F32, tag="ones_row")
            nc.gpsimd.memset(ones_row, 1.0)
            kmj = sb.tile([P, P], mybir.dt.int32, tag="kmj")   # k - m
            nc.gpsimd.iota(kmj, [[-1, P]], base=0, channel_multiplier=1)
            kmj2 = sb.tile([P, NCH], mybir.dt.int32, tag="kmj2")  # j - 2k - 2
            nc.gpsimd.iota(kmj2, [[1, NCH]], base=-2, channel_multiplier=-2)

            # PE warmup: dummy matmuls during the DMA window to raise the
            # tensor-engine p-state before the real transposes.
            if os.environ.get("KB_PEWARM", "0") == "1":
                dps = ps.tile([P, P], F32, tag="dps")
                for _wi in range(4):
                    nc.tensor.matmul(dps, ones_row, ones_row,
                                     start=True, stop=True)

            # Scalar: warm ACT so table loads overlap the DMA window.
            warm = sb.tile([P, 1], F32, tag="warm")
            nc.scalar.activation(warm, hpi_b, AF.Sin, bias=zero_b)

            # Vector pre-T0: tri/eye masks.
            tri_t = sb.tile([P, P], F32, tag="tri")     # tri[k,m]=1 iff k<m
            nc.vector.tensor_scalar(tri_t, kmj, 0, None, OP.is_lt)
            eye_t = sb.tile([P, P], F32, tag="eye")
            nc.vector.tensor_scalar(eye_t, kmj, 0, None, OP.is_equal)
            eye2 = sb.tile([P, NCH], F32, tag="eye2")
            nc.vector.tensor_scalar(eye2, kmj2, 0, None, OP.is_equal)

            # GpSimd after x0 sem: ne0' = -clip(x0_v, 0, 30)/dt (tiny ops)
            ne0p = sb.tile([1, 2], F32, tag="ne0p")
            nc.gpsimd.tensor_scalar(ne0p[0:1, 0:1], xrow[0:1, 3:4],
                                    0.0, 30.0, OP.max, OP.min)
            nc.gpsimd.tensor_scalar_mul(ne0p[0:1, 1:2], ne0p[0:1, 0:1],
                                        -1.0 / dt_f)
            ne0 = ne0p[0:1, 1:2]

            # ================= T0: U arrives =================
            # V critical head: fused accel clip -> scan input, steering clip,
            # chunk sums, the two clamp probes, probe-tail copies.
            nc.vector.scalar_tensor_tensor(d0v[:, 0:2 * C:2], Ut[:, 0:2 * C:2],
                                           -3.0, threes, OP.max, OP.min,
                                           accum_out=stage65[:, 0:1])  # s_p free
            # Both probes scan in place: slo's tail (-lo_p) lands at col 159,
            # then shi overwrites cols 0..127 leaving its tail (-hi_p) at 127.
            # Cols 128..158 hold dead slo values (bounded, never read as rows
            # 1..31 of the transpose are ignored).
            slo = big[:, 32:160]
            nc.vector.tensor_tensor_scan(slo, d0v, d1v, BIG, OP.subtract, OP.max)
            shi = big[:, 0:2 * C]
            nc.vector.tensor_tensor_scan(shi, d0v, d1v, -BIG, OP.subtract, OP.max)
            dcl = sb.tile([P, C], F32, tag="dcl")
            nc.vector.tensor_scalar(dcl, Ut[:, 1:2 * C:2], -0.6, 0.6, OP.max, OP.min)

            # S: sin/cos of clipped steering.
            sin_d = sb.tile([P, C], F32, tag="sin_d")
            nc.scalar.activation(sin_d, dcl, AF.Sin, bias=zero_b)
            cos_d = sb.tile([P, C], F32, tag="cos_d")
            nc.scalar.activation(cos_d, dcl, AF.Sin, bias=hpi_b)

            # PE: d0c (chunk sums scattered to even slots) directly in PSUM.
            d0c_ps = ps.tile([1, NCH], F32, tag="d0c_ps")
            nc.tensor.matmul(d0c_ps, stage65[:, 0:1], eye2, start=True, stop=True)
            # PE: transpose (-hi_p, -lo_p) to one-partition rows (33-col
            # lhsT window over big: -hi at psum row 0, -lo at row 32).
            rows = ps.tile([33, P], F32, tag="rows")
            nc.tensor.matmul(rows, big[:, 127:160], eye_t, start=True, stop=True)
            # Early halves of the offset accumulation groups: broadcast th0 /
            # (x00,y00) into the offg/offcd PSUM banks while PE is idle.
            offg = ps.tile([P, 1], F32, tag="offg")
            nc.tensor.matmul(offg, ones_row, xrow[0:1, 2:3], start=True, stop=False)
            offcd = ps.tile([P, 2], F32, tag="offcd")
            nc.tensor.matmul(offcd, ones_row, xrow[0:1, 0:2], start=True, stop=False)

            # Compose-scan input fills: d0c even <- s_p (S); d1c odd <- -hi_p,
            # d1c even <- +lo_p (both V; PSUM reads must start at quad rows).
            nc.vector.tensor_copy(d1c[0:1, 3:NCH:2], rows[0:1, :])
            nc.vector.tensor_scalar_mul(d1c[0:1, 2:NCH:2], rows[32:33, :], -1.0)

            # V: compose scan over the 128 chunk maps (identity slot first).
            comp = sb.tile([1, NCH], F32, tag="comp")
            nc.vector.tensor_tensor_scan(comp, d0c_ps[0:1, :], d1c, ne0,
                                         OP.subtract, OP.max)
            # PE: transpose -e_p back to partitions.
            nec = ps.tile([P, 1], F32, tag="nec")
            nc.tensor.matmul(nec, comp[0:1, 1:2 * P:2], one_t,
                             start=True, stop=True)
            # V in the matmul gap: tan(delta) pieces (2.8x-faster reciprocal).
            rcos = sb.tile([P, C], F32, tag="rcos")
            rscr = sb.tile([P, C], F32, tag="rscr")
            nc.vector.reciprocal_approx_accurate(rcos, cos_d, rscr)
            # ntanl = -tan(delta)*dt^2/L (sign folds against -w' scan out)
            ntanl = sb.tile([P, C], F32, tag="ntanl")
            nc.vector.scalar_tensor_tensor(ntanl, sin_d, NDT2 / L, rcos,
                                           OP.mult, OP.mult)
            # V: speed scan pass 2 (odd slots = -w'_{t+1}).
            sv = sb.tile([P, 2 * C], F32, tag="sv")
            nc.vector.tensor_tensor_scan(sv, d0v, d1v, nec[:, 0:1],
                                         OP.subtract, OP.max)

            OUT = sb.tile([P, 4 * C], F32, tag="OUT")
            # S: w column (w = -dt * sv_odd).
            nc.scalar.activation(OUT[:, 3:4 * C:4], sv[:, 1:2 * C:2],
                                 AF.Copy, scale=-dt_f)

            # theta increments g = w'*tan(delta)*dt^2/L = sv_odd*ntanl
            g = sb.tile([P, C], F32, tag="g")
            gs = sb.tile([P, 2], F32, tag="gs")
            nc.vector.tensor_tensor(g[:, 0:1], nec[:, 0:1], ntanl[:, 0:1],
                                    OP.mult)
            nc.vector.scalar_tensor_tensor(g[:, 1:C], sv[:, 1:2 * C - 2:2],
                                           1.0, ntanl[:, 1:C], OP.mult, OP.mult,
                                           accum_out=gs[:, 0:1])
            nc.vector.tensor_tensor(gs[:, 1:2], gs[:, 0:1], g[:, 0:1], OP.add)
            sg = sb.tile([P, C], F32, tag="sg")
            nc.vector.tensor_tensor_scan(sg, g, g, 0.0, OP.add, OP.bypass)
            # PE: theta chunk offsets from the fused sum — overlaps the sg scan.
            nc.tensor.matmul(offg, tri_t, gs[:, 1:2], start=False, stop=True)

            # V during offg matmul: w_dt (w*dt = -dt^2 * (-w')).
            w_dt = sb.tile([P, C], F32, tag="w_dt")
            nc.vector.tensor_scalar_mul(w_dt[:, 1:C], sv[:, 1:2 * C - 2:2], NDT2)
            nc.vector.tensor_scalar_mul(w_dt[:, 0:1], nec[:, 0:1], NDT2)

            # V: th_in = (sg + offg) - g   (theta at step start)
            th_in = sb.tile([P, C], F32, tag="th_in")
            nc.vector.scalar_tensor_tensor(th_in, sg, offg[:, 0:1], g,
                                           OP.add, OP.subtract)
            # S: theta column (inclusive) = sg + offg (SBUF bias via S copy).
            ob = sb.tile([P, 1], F32, tag="ob")
            nc.scalar.activation(ob, offg[:, 0:1], AF.Copy)
            nc.scalar.activation(OUT[:, 2:4 * C:4], sg, AF.Identity,
                                 bias=ob)

            # V: range reduce th_in to [-pi,pi], then wrapped +pi/2 copy.
            q = sb.tile([P, C], F32, tag="q")
            nc.vector.tensor_scalar(q, th_in, INV2PI, MAGIC, OP.mult, OP.add)
            kq = sb.tile([P, C], F32, tag="kq")
            nc.vector.tensor_scalar_add(kq, q, -MAGIC)
            trx = sb.tile([P, 2 * C], F32, tag="trx")
            nc.vector.cody_waite_cascade(trx[:, 0:C], th_in, kq, CW1, CW2, CW3)
            nc.vector.add_range_wrap(trx[:, C:2 * C], trx[:, 0:C], HPI, PI, TWOPI)
            # S: two Sins — sin half first so the d-mult overlaps the cos.
            sc = sb.tile([P, 2 * C], F32, tag="sc")
            sin_t = sc[:, 0:C]
            cos_t = sc[:, C:2 * C]
            nc.scalar.activation(sin_t, trx[:, 0:C], AF.Sin, bias=zero_b)
            nc.scalar.activation(cos_t, trx[:, C:2 * C], AF.Sin, bias=zero_b)

            # positions: increments with fused chunk sums, then prefix scans
            # seeded by the offset matmul writing straight into OUT.
            cd_s = sb.tile([P, 2], F32, tag="cd_s")
            d = sb.tile([P, C], F32, tag="d")
            nc.vector.scalar_tensor_tensor(d, w_dt, 1.0, sin_t,
                                           OP.mult, OP.mult,
                                           accum_out=cd_s[:, 1:2])
            c = sb.tile([P, C], F32, tag="c")
            nc.vector.scalar_tensor_tensor(c, w_dt, 1.0, cos_t,
                                           OP.mult, OP.mult,
                                           accum_out=cd_s[:, 0:1])
            # PE: position chunk offsets (x00,y00 already in the bank).
            nc.tensor.matmul(offcd, tri_t, cd_s, start=False, stop=True)
            nc.vector.tensor_tensor_scan(OUT[:, 0:4 * C:4], c, c,
                                         offcd[:, 0:1], OP.add, OP.bypass)
            nc.vector.tensor_tensor_scan(OUT[:, 1:4 * C:4], d, d,
                                         offcd[:, 1:2], OP.add, OP.bypass)

            # ---- stores (two halves drain on parallel queue sets) ----
            nc.sync.dma_start(
                out=out_d[1:HH + 1, :].rearrange("(p j) c -> p (j c)", p=P // 2),
                in_=OUT[0:P // 2, :])
            nc.scalar.dma_start(
                out=out_d[HH + 1:H + 1, :].rearrange("(p j) c -> p (j c)", p=P // 2),
                in_=OUT[P // 2:P, :])
            nc.sync.dma_start(out=out_d[0:1, 0:4], in_=xrow[0:1, 0:4])

    nc.compile()
    return nc


def kernel(x0, U, dt):
    key = float(np.asarray(dt, np.float32).reshape(())[()])
    if key not in _CACHE:
        _maybe_disable_io_dge()
        _CACHE[key] = _build(key)
    nc = _CACHE[key]

    in_map = {
        "x0": np.ascontiguousarray(np.asarray(x0, np.float32)),
        "U": np.ascontiguousarray(np.asarray(U, np.float32)),
    }
    in_maps = [in_map for _ in range(N_CORES)]

    trace = os.environ.get("KB_TRACE", "0") == "1"
    res = run_bass_kernel_spmd(nc, in_maps, list(range(N_CORES)), trace=trace)

    LAST_RUN_INFO.clear()
    LAST_RUN_INFO["exec_time_ns"] = res.exec_time_ns
    if res.instructions_and_trace is not None:
        LAST_RUN_INFO["trace_path"] = res.instructions_and_trace[1]

    return np.asarray(res.results[0]["out"], np.float32).reshape(H + 1, 4)

